# revision 21
# baseline (speedup 1.0000x reference)
"""Multi-head attention (B=4, S=2048, D=1024, 16 heads) on 8 TRN2 NeuronCores.

Sharding: data-parallel over batch (4) x tensor-parallel over heads (2 groups
of 8).  Core c handles batch c//2, head-group c%2.  Each core computes its
head-group's attention and the partial output projection through its slice of
Wo; the host sums the two partials per batch and adds bo.

Per-core kernel (all matmuls bf16, fp32 accumulation):
  - x^T built on-device via PE transposes (inputs cast fp32->bf16 during DMA)
  - Q^T,K^T = (x Wq/Wk)^T via transposed projections; V natural, augmented
    with a ones column per head (yields softmax denominators for free)
  - scores^T = K_h Q_h^T per head, two heads row-packed in the 128-wide PE
  - softmax without max subtraction: exp on ACT (scale=1/8 folded in),
    mask applied as multiply-by-complement (complement prepared once:
    int32 -> bf16 (m==0) on DVE, bounced through DRAM, read back with the
    DMA xbar transpose)
  - out_h^T = V_aug^T P^T accumulated on PE; row 64 = denominator
  - normalize with DVE reciprocal + GPSIMD partition_broadcast
  - partial out = X Wo_slice in natural layout, DMA'd out in fp32
"""

import sys

if "/opt/trn_rl_repo" not in sys.path:
    sys.path.insert(0, "/opt/trn_rl_repo")

from contextlib import ExitStack

import numpy as np

import concourse.bass as bass
import concourse.tile as tile
from concourse import mybir
from concourse.masks import make_identity

FP32 = mybir.dt.float32
BF16 = mybir.dt.bfloat16
INT32 = mybir.dt.int32

# Full-problem constants (per core shard)
S_FULL = 2048
D_FULL = 1024
DML_FULL = 512  # local d_model slice = 8 heads * 64
DK = 64
N_CORES = 8


def build_attention(tc: tile.TileContext, io, S, D, DML):
    """Emit the per-core attention program.

    io: dict of DRAM APs: xq,xk,xv [S,D] f32; mask [S,S] i32; wq,wk,wv [D,DML]
    f32; wo [DML,D] f32; bq,bk,bv [DML] f32; out [S,D] f32.
    """
    nc = tc.nc
    ctx = ExitStack()

    HG = DML // DK          # local heads
    NPAIR = HG // 2
    P = 128
    nS = S // P             # sequence tiles
    nD = D // P             # d_input tiles
    nDM = DML // P          # local d_model tiles (= head pairs)
    SQB = min(512, S)       # sq block (psum free width)
    nSQB = S // SQB
    OCT = min(8, nS)        # s-tiles per transpose group
    NOUT = min(512, D)      # out-proj free width
    VST = DK + 2            # V_aug per-head stride (64 data + ones + pad)
    assert nS % OCT == 0 and HG % 2 == 0

    with ctx:
        # ---------------- pools ----------------
        singles = ctx.enter_context(tc.tile_pool(name="singles", bufs=1))
        w_pool = ctx.enter_context(tc.tile_pool(name="w", bufs=1))
        wo_pool = ctx.enter_context(tc.tile_pool(name="wo", bufs=1))
        mstage = ctx.enter_context(tc.tile_pool(name="mstage", bufs=2))
        xe_pool = ctx.enter_context(tc.tile_pool(name="xe", bufs=OCT))
        xt_pool = ctx.enter_context(tc.tile_pool(name="xt", bufs=1))
        qt_pool = ctx.enter_context(tc.tile_pool(name="qt", bufs=1))
        kt_pool = ctx.enter_context(tc.tile_pool(name="kt", bufs=1))
        va_pool = ctx.enter_context(tc.tile_pool(name="va", bufs=1))
        xo_pool = ctx.enter_context(tc.tile_pool(name="xo", bufs=1))
        mt_pool = ctx.enter_context(tc.tile_pool(name="mt", bufs=min(12, 2 * (nS // 2))))  # single tag
        p_pool = ctx.enter_context(tc.tile_pool(name="p", bufs=4))
        rd_pool = ctx.enter_context(tc.tile_pool(name="rd", bufs=4))
        out_pool = ctx.enter_context(tc.tile_pool(name="out", bufs=4))
        dram = ctx.enter_context(tc.tile_pool(name="dram", bufs=1, space="DRAM"))

        s_psum = ctx.enter_context(tc.tile_pool(name="s_psum", bufs=2, space="PSUM"))
        pv_psum = ctx.enter_context(tc.tile_pool(name="pv_psum", bufs=2, space="PSUM"))
        m_psum = ctx.enter_context(tc.tile_pool(name="m_psum", bufs=2, space="PSUM"))

        # ---------------- constants ----------------
        identity = singles.tile([P, P], FP32)
        make_identity(nc, identity)
        ones_row = singles.tile([1, P], BF16)
        nc.vector.memset(ones_row, 1.0)
        bq_sb = singles.tile([P, nDM], FP32)
        bk_sb = singles.tile([P, nDM], FP32)
        for b_sb, b_ap in ((bq_sb, io["bq"]), (bk_sb, io["bk"])):
            src = bass.AP(tensor=b_ap.tensor, offset=b_ap.offset, ap=[[1, P], [P, nDM]])
            nc.sync.dma_start(out=b_sb, in_=src)
        bv_sb = singles.tile([1, DML], BF16)
        nc.gpsimd.dma_start(out=bv_sb, in_=io["bv"][None, :])

        # ---------------- mask complement -> DRAM (bf16), per sq block ------
        # one DRAM tile per sq block so block j's transposed reads only wait
        # on block j's writes; blocks j>=1 are prepared during attention.
        maskc = [dram.tile([SQB, S], BF16, tag=f"mc{j}", name=f"mc{j}")
                 for j in range(nSQB)]
        MH = 1024 if S % 1024 == 0 else S

        def mask_prep(j):
            for r in range(SQB // P):
                row0 = j * SQB + r * P
                for ch in range(S // MH):
                    mi = mstage.tile([P, MH], INT32, tag="mi")
                    nc.sync.dma_start(out=mi, in_=io["mask"][row0:row0 + P, ch * MH:(ch + 1) * MH])
                    mo = xe_pool.tile([P, MH], BF16, tag="xe")
                    # (mask == 0) -> 1.0 else 0.0
                    nc.vector.tensor_scalar(mo, mi, 0, None, mybir.AluOpType.is_equal)
                    nc.sync.dma_start(out=maskc[j][r * P:(r + 1) * P, ch * MH:(ch + 1) * MH], in_=mo)

        mask_prep(0)

        # ---------------- x^T + projections ----------------
        def build_xt(x_dram):
            """Load x [S,D] f32, transpose on PE, return nD bf16 tiles [128, S]."""
            xt = [xt_pool.tile([P, S], BF16, tag=f"xt{dj}", name=f"xt{dj}") for dj in range(nD)]
            for a in range(nS // OCT):
                xe = []
                for ii in range(OCT):
                    t = xe_pool.tile([P, D], FP32, tag="xe")
                    nc.sync.dma_start(out=t, in_=x_dram[(a * OCT + ii) * P:(a * OCT + ii + 1) * P, :])
                    xe.append(t)
                for dj in range(nD):
                    tp = s_psum.tile([P, OCT * P], FP32, tag="s")
                    for ii in range(OCT):
                        nc.tensor.transpose(tp[:, ii * P:(ii + 1) * P],
                                            xe[ii][:, dj * P:(dj + 1) * P], identity)
                    eng = nc.vector if dj % 2 == 0 else nc.scalar
                    if eng is nc.vector:
                        nc.vector.tensor_copy(out=xt[dj][:, a * OCT * P:(a + 1) * OCT * P], in_=tp)
                    else:
                        nc.scalar.copy(xt[dj][:, a * OCT * P:(a + 1) * OCT * P], tp)
            return xt

        def load_w(w_dram):
            w = []
            for kt in range(nD):
                ws = mstage.tile([P, DML], FP32, tag="mi")
                nc.sync.dma_start(out=ws, in_=w_dram[kt * P:(kt + 1) * P, :])
                t = w_pool.tile([P, DML], BF16, tag=f"w{kt}")
                nc.vector.tensor_copy(t, ws)
                w.append(t)
            return w

        # Q^T, K^T: [dm, s] tiles
        qt = [qt_pool.tile([P, S], BF16, tag=f"qt{m}", name=f"qtt{m}") for m in range(nDM)]
        kt_t = [kt_pool.tile([P, S], BF16, tag=f"kt{m}", name=f"ktt{m}") for m in range(nDM)]
        for x_dram, w_dram, b_sb, dst in (
            (io["xq"], io["wq"], bq_sb, qt),
            (io["xk"], io["wk"], bk_sb, kt_t),
        ):
            xt = build_xt(x_dram)
            w = load_w(w_dram)
            for mj in range(nDM):
                for nb in range(nSQB):
                    ps = m_psum.tile([P, SQB], FP32, tag="m")
                    for kj in range(nD):
                        nc.tensor.matmul(ps, w[kj][:, mj * P:(mj + 1) * P],
                                         xt[kj][:, nb * SQB:(nb + 1) * SQB],
                                         start=(kj == 0), stop=(kj == nD - 1))
                    nc.scalar.activation(dst[mj][:, nb * SQB:(nb + 1) * SQB], ps,
                                         mybir.ActivationFunctionType.Identity,
                                         bias=b_sb[:, mj:mj + 1], scale=1.0)

        # V natural [s, dm] + ones column per head, bias added via rank-1 matmul
        va = [va_pool.tile([P, HG * VST], BF16, tag=f"va{si}", name=f"vat{si}") for si in range(nS)]
        xt = build_xt(io["xv"])
        w = load_w(io["wv"])
        for si in range(nS):
            ps = m_psum.tile([P, DML], FP32, tag="m")
            for kj in range(nD):
                nc.tensor.matmul(ps, xt[kj][:, si * P:(si + 1) * P], w[kj],
                                 start=(kj == 0), stop=False)
            nc.tensor.matmul(ps, ones_row, bv_sb, start=False, stop=True)
            va3 = va[si].rearrange("p (h e) -> p h e", e=VST)
            ps3 = ps.rearrange("p (h e) -> p h e", e=DK)
            nc.scalar.copy(va3[:, :, 0:DK], ps3)
            nc.vector.memset(va3[:, :, DK:DK + 1], 1.0)

        # ---------------- attention + output projection ----------------
        wo = []
        for kj in range(nDM):
            ws = mstage.tile([P, D], FP32, tag="mi")
            nc.sync.dma_start(out=ws, in_=io["wo"][kj * P:(kj + 1) * P, :])
            t = wo_pool.tile([P, D], BF16, tag=f"wo{kj}")
            nc.vector.tensor_copy(t, ws)
            wo.append(t)
        xo = [xo_pool.tile([P, S], BF16, tag=f"xo{m}", name=f"xot{m}") for m in range(nDM)]

        for j in range(nSQB):
            # transposed mask-complement tiles for this sq block
            mts = []
            for bt in range(nS // 2):
                mt = mt_pool.tile([P, 2 * SQB], BF16, tag="mt")
                for half in (0, 1):
                    k = 2 * bt + half
                    nc.sync.dma_start(
                        out=mt[:, half * SQB:(half + 1) * SQB],
                        in_=maskc[j][:, k * P:(k + 1) * P],
                        transpose=True)
                mts.append(mt)
            if j + 1 < nSQB:
                mask_prep(j + 1)

            for pr in range(NPAIR):
                pv = [pv_psum.tile([P, SQB], FP32, tag="pv", name=f"pv{j}_{pr}_{_}") for _ in range(2)]
                for bt in range(nS // 2):
                    sc = [s_psum.tile([P, 2 * SQB], FP32, tag="s", name=f"sc{j}_{pr}_{bt}_{_}") for _ in range(2)]
                    for half in (0, 1):
                        k = 2 * bt + half
                        for hh in (0, 1):
                            nc.tensor.matmul(
                                sc[hh][:, half * SQB:(half + 1) * SQB],
                                kt_t[pr][hh * DK:(hh + 1) * DK, k * P:(k + 1) * P],
                                qt[pr][hh * DK:(hh + 1) * DK, j * SQB:(j + 1) * SQB],
                                start=True, stop=True)
                    pp = []
                    for hh in (0, 1):
                        t = p_pool.tile([P, 2 * SQB], BF16, tag="p", name=f"pp{j}_{pr}_{bt}_{hh}")
                        nc.scalar.activation(t, sc[hh], mybir.ActivationFunctionType.Exp,
                                             scale=1.0 / np.sqrt(DK))
                        nc.vector.tensor_mul(t, t, mts[bt])
                        pp.append(t)
                    for half in (0, 1):
                        k = 2 * bt + half
                        for hh in (0, 1):
                            h = 2 * pr + hh
                            nc.tensor.matmul(
                                pv[hh][0:DK + 1, :],
                                va[k][:, h * VST:h * VST + DK + 1],
                                pp[hh][:, half * SQB:(half + 1) * SQB],
                                start=(k == 0), stop=(k == nS - 1))
                for hh in (0, 1):
                    rden = rd_pool.tile([1, SQB], BF16, tag="rden")
                    with nc.allow_low_precision(reason="softmax rdenom bcast in bf16"):
                        nc.vector.reciprocal(rden, pv[hh][DK:DK + 1, :])
                    # broadcast partition 0 -> DK partitions via rank-1 matmul
                    rp = m_psum.tile([P, SQB], FP32, tag="m")
                    nc.tensor.matmul(rp[0:DK, :], ones_row[:, 0:DK], rden,
                                     start=True, stop=True)
                    rdb = rd_pool.tile([DK, SQB], BF16, tag="rdb")
                    nc.vector.tensor_copy(rdb, rp[0:DK, :])
                    nc.vector.scalar_tensor_tensor(
                        out=xo[pr][hh * DK:(hh + 1) * DK, j * SQB:(j + 1) * SQB],
                        in0=pv[hh][0:DK, :], scalar=1.0, in1=rdb,
                        op0=mybir.AluOpType.bypass, op1=mybir.AluOpType.mult)

            # output projection for this sq block
            for st in range(SQB // P):
                for nb in range(D // NOUT):
                    wp = m_psum.tile([P, NOUT], FP32, tag="m")
                    for kj in range(nDM):
                        nc.tensor.matmul(
                            wp, xo[kj][:, j * SQB + st * P:j * SQB + (st + 1) * P],
                            wo[kj][:, nb * NOUT:(nb + 1) * NOUT],
                            start=(kj == 0), stop=(kj == nDM - 1))
                    ob = out_pool.tile([P, NOUT], FP32, tag="ob")
                    nc.vector.tensor_copy(ob, wp)
                    nc.sync.dma_start(
                        out=io["out"][j * SQB + st * P:j * SQB + (st + 1) * P,
                                      nb * NOUT:(nb + 1) * NOUT],
                        in_=ob)


def split_excess_waits(nc, default_limit=1, drain_limit=1, dma_limit=1):
    """The walrus build here rejects instructions with too many sem waits
    (Drain/CTRL takes 1).  Hoist excess waits onto same-engine NoOp carriers
    inserted immediately before the offender — semantically identical."""
    n_new = 0
    for f in nc.m.functions:
        for blk in f.blocks:
            insts = blk.instructions
            pos = 0
            while pos < len(insts):
                i = insts[pos]
                if isinstance(i, mybir.InstDrain):
                    limit = drain_limit
                elif isinstance(i, (mybir.InstDMACopy, mybir.InstDmaTransposeAnt)):
                    limit = dma_limit
                else:
                    limit = default_limit
                si = getattr(i, "sync_info", None)
                if si is not None and si.on_wait is not None and len(si.on_wait) > limit:
                    excess = []
                    while len(si.on_wait) > limit:
                        excess.append(si.on_wait.pop())
                    carriers = []
                    for j in range(0, len(excess), max(default_limit, 1)):
                        nd = mybir.InstNoOp(name=f"I-sw{n_new}", ins=[], outs=[])
                        n_new += 1
                        nd.engine = i.engine
                        nd.sync_info = mybir.SyncInfo(
                            on_wait=excess[j:j + default_limit], on_update=[])
                        carriers.append(nd)
                    for k, nd in enumerate(carriers):
                        insts.insert(pos + k, nd)
                    pos += len(carriers)
                pos += 1
    return n_new


def build_nc(S=S_FULL, D=D_FULL, DML=DML_FULL):
    nc = bass.Bass("TRN2", target_bir_lowering=False, debug=False, num_devices=N_CORES)
    io = {
        "xq": nc.dram_tensor("xq", [S, D], FP32, kind="ExternalInput")[:],
        "xk": nc.dram_tensor("xk", [S, D], FP32, kind="ExternalInput")[:],
        "xv": nc.dram_tensor("xv", [S, D], FP32, kind="ExternalInput")[:],
        "mask": nc.dram_tensor("mask", [S, S], INT32, kind="ExternalInput")[:],
        "wq": nc.dram_tensor("wq", [D, DML], FP32, kind="ExternalInput")[:],
        "wk": nc.dram_tensor("wk", [D, DML], FP32, kind="ExternalInput")[:],
        "wv": nc.dram_tensor("wv", [D, DML], FP32, kind="ExternalInput")[:],
        "wo": nc.dram_tensor("wo", [DML, D], FP32, kind="ExternalInput")[:],
        "bq": nc.dram_tensor("bq", [DML], FP32, kind="ExternalInput")[:],
        "bk": nc.dram_tensor("bk", [DML], FP32, kind="ExternalInput")[:],
        "bv": nc.dram_tensor("bv", [DML], FP32, kind="ExternalInput")[:],
        "out": nc.dram_tensor("out", [S, D], FP32, kind="ExternalOutput")[:],
    }
    with tile.TileContext(nc) as tc:
        build_attention(tc, io, S, D, DML)
    split_excess_waits(nc)
    return nc


_NC_CACHE = {}


def kernel(**inputs):
    query = np.asarray(inputs["query"], np.float32)
    key = np.asarray(inputs["key"], np.float32)
    value = np.asarray(inputs["value"], np.float32)
    mask = np.asarray(inputs["mask"], np.int32)
    Wq, bq = np.asarray(inputs["Wq"], np.float32), np.asarray(inputs["bq"], np.float32)
    Wk, bk = np.asarray(inputs["Wk"], np.float32), np.asarray(inputs["bk"], np.float32)
    Wv, bv = np.asarray(inputs["Wv"], np.float32), np.asarray(inputs["bv"], np.float32)
    Wo, bo = np.asarray(inputs["Wo"], np.float32), np.asarray(inputs["bo"], np.float32)

    B = query.shape[0]
    DML = Wq.shape[1] // 2  # head-group slice width

    if "nc" not in _NC_CACHE:
        _NC_CACHE["nc"] = build_nc()
    nc = _NC_CACHE["nc"]

    in_maps = []
    for c in range(N_CORES):
        b, g = divmod(c, 2)
        sl = slice(g * DML, (g + 1) * DML)
        in_maps.append({
            "xq": np.ascontiguousarray(query[b]),
            "xk": np.ascontiguousarray(key[b]),
            "xv": np.ascontiguousarray(value[b]),
            "mask": np.ascontiguousarray(mask[b]),
            "wq": np.ascontiguousarray(Wq[:, sl]),
            "wk": np.ascontiguousarray(Wk[:, sl]),
            "wv": np.ascontiguousarray(Wv[:, sl]),
            "wo": np.ascontiguousarray(Wo[sl, :]),
            "bq": np.ascontiguousarray(bq[sl]),
            "bk": np.ascontiguousarray(bk[sl]),
            "bv": np.ascontiguousarray(bv[sl]),
        })

    import os

    from concourse.bass_utils import run_bass_kernel_spmd
    trace = os.environ.get("KERNEL_TRACE", "0") == "1"
    res = run_bass_kernel_spmd(nc, in_maps, core_ids=list(range(N_CORES)), trace=trace)
    _NC_CACHE["last_result"] = res
    out = np.stack([
        res.results[2 * b]["out"] + res.results[2 * b + 1]["out"] + bo
        for b in range(B)
    ]).astype(np.float32)
    return out


# revision 29
# speedup vs baseline: 1.4763x; 1.4763x over previous
"""Multi-head attention (B=4, S=2048, D=1024, 16 heads) on 8 TRN2 NeuronCores.

Sharding: data-parallel over batch (4) x tensor-parallel over heads (2 groups
of 8).  Core c handles batch c//2, head-group c%2.  Each core computes its
head-group's attention and the partial output projection through its slice of
Wo; the host sums the two partials per batch and adds bo.

Per-core kernel (all matmuls bf16, fp32 accumulation):
  - x^T built on-device via PE transposes (inputs cast fp32->bf16 by the
    SWDGE DMA on the otherwise-idle GPSIMD engine)
  - Q^T,K^T = (x Wq/Wk)^T via transposed projections; V natural, augmented
    with a ones column per head (yields softmax denominators for free)
  - scores^T = K_h Q_h^T per head, two heads row-packed in the 128-wide PE
  - softmax without max subtraction (scores ~ N(0,1), no overflow): exp on
    ACT with the 1/sqrt(dk) scale folded in, mask applied as a bf16
    multiply-by-complement on DVE; the complement (mask==0) is produced per
    sq-block just in time, bounced through DRAM, and read back with the DMA
    xbar transpose so mask DMA streams during attention
  - out_h^T = V_aug^T P^T accumulated on PE; row 64 = denominator
  - normalize: DVE reciprocal + rank-1 PE broadcast matmul + DVE
    scalar_tensor_tensor (fused PSUM read, scale, bf16 store)
  - partial out = X Wo_slice in natural layout, DMA'd out in fp32

The walrus build here accepts at most one semaphore wait per instruction;
split_excess_waits() hoists extra waits onto NoOp carriers post-scheduling.
"""

import sys

if "/opt/trn_rl_repo" not in sys.path:
    sys.path.insert(0, "/opt/trn_rl_repo")

from contextlib import ExitStack

import numpy as np

import concourse.bass as bass
import concourse.tile as tile
from concourse import mybir
from concourse.masks import make_identity

FP32 = mybir.dt.float32
BF16 = mybir.dt.bfloat16
INT32 = mybir.dt.int32

# Full-problem constants (per core shard)
S_FULL = 2048
D_FULL = 1024
DML_FULL = 512  # local d_model slice = 8 heads * 64
DK = 64
N_CORES = 8


def build_attention(tc: tile.TileContext, io, S, D, DML):
    """Emit the per-core attention program.

    io: dict of DRAM APs: xq,xk,xv [S,D] f32; mask [S,S] i32; wq,wk,wv [D,DML]
    f32; wo [DML,D] f32; bq,bk,bv [DML] f32; out [S,D] f32.
    """
    nc = tc.nc
    ctx = ExitStack()

    HG = DML // DK          # local heads
    NPAIR = HG // 2
    P = 128
    nS = S // P             # sequence tiles
    nD = D // P             # d_input tiles
    nDM = DML // P          # local d_model tiles (= head pairs)
    SQB = min(512, S)       # sq block (psum free width)
    nSQB = S // SQB
    OCT = min(8, nS)        # s-tiles per transpose group
    NOUT = min(512, D)      # out-proj free width
    VST = DK + 2            # V_aug per-head stride (64 data + ones + pad)
    assert nS % OCT == 0 and HG % 2 == 0

    with ctx:
        # ---------------- pools ----------------
        singles = ctx.enter_context(tc.tile_pool(name="singles", bufs=1))
        w_pool = ctx.enter_context(tc.tile_pool(name="w", bufs=1))
        wo_pool = ctx.enter_context(tc.tile_pool(name="wo", bufs=1))
        mstage = ctx.enter_context(tc.tile_pool(name="mstage", bufs=2))
        xe_pool = ctx.enter_context(tc.tile_pool(name="xe", bufs=OCT))
        xt_pool = ctx.enter_context(tc.tile_pool(name="xt", bufs=1))
        qt_pool = ctx.enter_context(tc.tile_pool(name="qt", bufs=1))
        kt_pool = ctx.enter_context(tc.tile_pool(name="kt", bufs=1))
        va_pool = ctx.enter_context(tc.tile_pool(name="va", bufs=1))
        xo_pool = ctx.enter_context(tc.tile_pool(name="xo", bufs=1))
        mt_pool = ctx.enter_context(tc.tile_pool(name="mt", bufs=min(12, 2 * (nS // 2))))
        p_pool = ctx.enter_context(tc.tile_pool(name="p", bufs=6))
        rd_pool = ctx.enter_context(tc.tile_pool(name="rd", bufs=4))
        out_pool = ctx.enter_context(tc.tile_pool(name="out", bufs=4))
        dram = ctx.enter_context(tc.tile_pool(name="dram", bufs=1, space="DRAM"))

        s_psum = ctx.enter_context(tc.tile_pool(name="s_psum", bufs=2, space="PSUM"))
        pv_psum = ctx.enter_context(tc.tile_pool(name="pv_psum", bufs=2, space="PSUM"))
        m_psum = ctx.enter_context(tc.tile_pool(name="m_psum", bufs=2, space="PSUM"))

        # ---------------- constants ----------------
        identity = singles.tile([P, P], BF16)
        make_identity(nc, identity)
        ones_row = singles.tile([1, P], BF16)
        nc.vector.memset(ones_row, 1.0)
        bq_sb = singles.tile([P, nDM], FP32)
        bk_sb = singles.tile([P, nDM], FP32)
        for b_sb, b_ap in ((bq_sb, io["bq"]), (bk_sb, io["bk"])):
            src = bass.AP(tensor=b_ap.tensor, offset=b_ap.offset, ap=[[1, P], [P, nDM]])
            nc.sync.dma_start(out=b_sb, in_=src)
        bv_sb = singles.tile([1, DML], BF16)
        nc.gpsimd.dma_start(out=bv_sb, in_=io["bv"][None, :])

        # ---------------- mask complement -> DRAM (bf16), per sq block ------
        maskc = [dram.tile([SQB, S], BF16, tag=f"mc{j}", name=f"mc{j}")
                 for j in range(nSQB)]
        MH = 1024 if S % 1024 == 0 else S

        def mask_prep_c(j):
            for r in range(SQB // P):
                row0 = j * SQB + r * P
                for ch in range(S // MH):
                    mi = mstage.tile([P, MH], INT32, tag="mi")
                    nc.sync.dma_start(out=mi, in_=io["mask"][row0:row0 + P, ch * MH:(ch + 1) * MH])
                    mo = xe_pool.tile([P, MH], BF16, tag="xe")
                    # (mask == 0) -> 1.0 else 0.0
                    nc.vector.tensor_scalar(mo, mi, 0, None, mybir.AluOpType.is_equal)
                    nc.sync.dma_start(out=maskc[j][r * P:(r + 1) * P, ch * MH:(ch + 1) * MH], in_=mo)

        def mask_prep(j):
            mts = [mt_pool.tile([P, 2 * SQB], BF16, tag="mt", name=f"mt{j}_{bt}")
                   for bt in range(nS // 2)]
            for bt in range(nS // 2):
                for half in (0, 1):
                    k = 2 * bt + half
                    nc.sync.dma_start(
                        out=mts[bt][:, half * SQB:(half + 1) * SQB],
                        in_=maskc[j][:, k * P:(k + 1) * P],
                        transpose=True)
            return mts

        # ---------------- x^T + projections ----------------
        def build_xt(x_dram):
            """Load x [S,D] f32, transpose on PE, return nD bf16 tiles [128, S]."""
            xt = [xt_pool.tile([P, S], BF16, tag=f"xt{dj}", name=f"xt{dj}") for dj in range(nD)]
            for a in range(nS // OCT):
                xe = []
                for ii in range(OCT):
                    t = xe_pool.tile([P, D], BF16, tag="xe2")
                    nc.gpsimd.dma_start(out=t, in_=x_dram[(a * OCT + ii) * P:(a * OCT + ii + 1) * P, :])
                    xe.append(t)
                for dj in range(nD):
                    tp = s_psum.tile([P, OCT * P], BF16, tag="s")
                    for ii in range(OCT):
                        nc.tensor.transpose(tp[:, ii * P:(ii + 1) * P],
                                            xe[ii][:, dj * P:(dj + 1) * P], identity)
                    eng = nc.vector if dj % 2 == 0 else nc.scalar
                    if eng is nc.vector:
                        nc.vector.tensor_copy(out=xt[dj][:, a * OCT * P:(a + 1) * OCT * P], in_=tp)
                    else:
                        nc.scalar.copy(xt[dj][:, a * OCT * P:(a + 1) * OCT * P], tp)
            return xt

        def load_w(w_dram):
            w = []
            for kt in range(nD):
                ws = mstage.tile([P, DML], FP32, tag="mi")
                nc.sync.dma_start(out=ws, in_=w_dram[kt * P:(kt + 1) * P, :])
                t = w_pool.tile([P, DML], BF16, tag=f"w{kt}")
                nc.vector.tensor_copy(t, ws)
                w.append(t)
            return w

        # Q^T, K^T: [dm, s] tiles
        qt = [qt_pool.tile([P, S], BF16, tag=f"qt{m}", name=f"qtt{m}") for m in range(nDM)]
        kt_t = [kt_pool.tile([P, S], BF16, tag=f"kt{m}", name=f"ktt{m}") for m in range(nDM)]
        for x_dram, w_dram, b_sb, dst in (
            (io["xq"], io["wq"], bq_sb, qt),
            (io["xk"], io["wk"], bk_sb, kt_t),
        ):
            xt = build_xt(x_dram)
            w = load_w(w_dram)
            for mj in range(nDM):
                for nb in range(nSQB):
                    ps = m_psum.tile([P, SQB], FP32, tag="m")
                    for kj in range(nD):
                        nc.tensor.matmul(ps, w[kj][:, mj * P:(mj + 1) * P],
                                         xt[kj][:, nb * SQB:(nb + 1) * SQB],
                                         start=(kj == 0), stop=(kj == nD - 1))
                    nc.scalar.activation(dst[mj][:, nb * SQB:(nb + 1) * SQB], ps,
                                         mybir.ActivationFunctionType.Identity,
                                         bias=b_sb[:, mj:mj + 1], scale=1.0)

        # V natural [s, dm] + ones column per head, bias added via rank-1 matmul
        va = [va_pool.tile([P, HG * VST], BF16, tag=f"va{si}", name=f"vat{si}") for si in range(nS)]
        xt = build_xt(io["xv"])
        w = load_w(io["wv"])
        for si in range(nS):
            ps = m_psum.tile([P, DML], FP32, tag="m")
            for kj in range(nD):
                nc.tensor.matmul(ps, xt[kj][:, si * P:(si + 1) * P], w[kj],
                                 start=(kj == 0), stop=False)
            nc.tensor.matmul(ps, ones_row, bv_sb, start=False, stop=True)
            va3 = va[si].rearrange("p (h e) -> p h e", e=VST)
            ps3 = ps.rearrange("p (h e) -> p h e", e=DK)
            nc.scalar.copy(va3[:, :, 0:DK], ps3)
            nc.vector.memset(va3[:, :, DK:DK + 1], 1.0)

        # ---------------- attention + output projection ----------------
        wo = []
        for kj in range(nDM):
            ws = mstage.tile([P, D], FP32, tag="mi")
            nc.sync.dma_start(out=ws, in_=io["wo"][kj * P:(kj + 1) * P, :])
            t = wo_pool.tile([P, D], BF16, tag=f"wo{kj}")
            nc.vector.tensor_copy(t, ws)
            wo.append(t)
        xo = [xo_pool.tile([P, S], BF16, tag=f"xo{m}", name=f"xot{m}") for m in range(nDM)]

        mask_prep_c(0)
        mts_all = {0: mask_prep(0)}
        for j in range(nSQB):
            mts = mts_all.pop(j)
            if j + 1 < nSQB:
                mask_prep_c(j + 1)
                mts_all[j + 1] = mask_prep(j + 1)

            for pr in range(NPAIR):
                pv = [pv_psum.tile([P, SQB], FP32, tag="pv", name=f"pv{j}_{pr}_{_}") for _ in range(2)]
                for bt in range(nS // 2):
                    sc = [s_psum.tile([P, 2 * SQB], FP32, tag="s", name=f"sc{j}_{pr}_{bt}_{_}") for _ in range(2)]
                    for half in (0, 1):
                        k = 2 * bt + half
                        for hh in (0, 1):
                            nc.tensor.matmul(
                                sc[hh][:, half * SQB:(half + 1) * SQB],
                                kt_t[pr][hh * DK:(hh + 1) * DK, k * P:(k + 1) * P],
                                qt[pr][hh * DK:(hh + 1) * DK, j * SQB:(j + 1) * SQB],
                                start=True, stop=True)
                    pp = []
                    for hh in (0, 1):
                        t = p_pool.tile([P, 2 * SQB], BF16, tag="p", name=f"pp{j}_{pr}_{bt}_{hh}")
                        nc.scalar.activation(t, sc[hh], mybir.ActivationFunctionType.Exp,
                                             scale=1.0 / np.sqrt(DK))
                        nc.vector.tensor_mul(t, t, mts[bt])
                        pp.append(t)
                    for half in (0, 1):
                        k = 2 * bt + half
                        for hh in (0, 1):
                            h = 2 * pr + hh
                            nc.tensor.matmul(
                                pv[hh][0:DK + 1, :],
                                va[k][:, h * VST:h * VST + DK + 1],
                                pp[hh][:, half * SQB:(half + 1) * SQB],
                                start=(k == 0), stop=(k == nS - 1))
                for hh in (0, 1):
                    rden = rd_pool.tile([1, SQB], BF16, tag="rden")
                    with nc.allow_low_precision(reason="softmax rdenom bcast in bf16"):
                        nc.vector.reciprocal(rden, pv[hh][DK:DK + 1, :])
                    # broadcast partition 0 -> DK partitions via rank-1 matmul
                    rp = m_psum.tile([P, SQB], FP32, tag="m")
                    nc.tensor.matmul(rp[0:DK, :], ones_row[:, 0:DK], rden,
                                     start=True, stop=True)
                    rdb = rd_pool.tile([DK, SQB], BF16, tag="rdb")
                    nc.vector.tensor_copy(rdb, rp[0:DK, :])
                    nc.vector.scalar_tensor_tensor(
                        out=xo[pr][hh * DK:(hh + 1) * DK, j * SQB:(j + 1) * SQB],
                        in0=pv[hh][0:DK, :], scalar=1.0, in1=rdb,
                        op0=mybir.AluOpType.bypass, op1=mybir.AluOpType.mult)

            # output projection for this sq block
            for st in range(SQB // P):
                for nb in range(D // NOUT):
                    wp = m_psum.tile([P, NOUT], FP32, tag="m")
                    for kj in range(nDM):
                        nc.tensor.matmul(
                            wp, xo[kj][:, j * SQB + st * P:j * SQB + (st + 1) * P],
                            wo[kj][:, nb * NOUT:(nb + 1) * NOUT],
                            start=(kj == 0), stop=(kj == nDM - 1))
                    ob = out_pool.tile([P, NOUT], FP32, tag="ob")
                    nc.vector.tensor_copy(ob, wp)
                    nc.sync.dma_start(
                        out=io["out"][j * SQB + st * P:j * SQB + (st + 1) * P,
                                      nb * NOUT:(nb + 1) * NOUT],
                        in_=ob)


def split_excess_waits(nc, default_limit=1, drain_limit=1, dma_limit=1):
    """The walrus build here rejects instructions with too many sem waits
    (Drain/CTRL takes 1).  Hoist excess waits onto same-engine NoOp carriers
    inserted immediately before the offender — semantically identical."""
    n_new = 0
    for f in nc.m.functions:
        for blk in f.blocks:
            insts = blk.instructions
            pos = 0
            while pos < len(insts):
                i = insts[pos]
                if isinstance(i, mybir.InstDrain):
                    limit = drain_limit
                elif isinstance(i, (mybir.InstDMACopy, mybir.InstDmaTransposeAnt)):
                    limit = dma_limit
                else:
                    limit = default_limit
                si = getattr(i, "sync_info", None)
                if si is not None and si.on_wait is not None and len(si.on_wait) > limit:
                    excess = []
                    while len(si.on_wait) > limit:
                        excess.append(si.on_wait.pop())
                    carriers = []
                    for j in range(0, len(excess), max(default_limit, 1)):
                        nd = mybir.InstNoOp(name=f"I-sw{n_new}", ins=[], outs=[])
                        n_new += 1
                        nd.engine = i.engine
                        nd.sync_info = mybir.SyncInfo(
                            on_wait=excess[j:j + default_limit], on_update=[])
                        carriers.append(nd)
                    for k, nd in enumerate(carriers):
                        insts.insert(pos + k, nd)
                    pos += len(carriers)
                pos += 1
    return n_new


def build_nc(S=S_FULL, D=D_FULL, DML=DML_FULL, reps=1):
    nc = bass.Bass("TRN2", target_bir_lowering=False, debug=False, num_devices=N_CORES)
    io = {
        "xq": nc.dram_tensor("xq", [S, D], FP32, kind="ExternalInput")[:],
        "xk": nc.dram_tensor("xk", [S, D], FP32, kind="ExternalInput")[:],
        "xv": nc.dram_tensor("xv", [S, D], FP32, kind="ExternalInput")[:],
        "mask": nc.dram_tensor("mask", [S, S], INT32, kind="ExternalInput")[:],
        "wq": nc.dram_tensor("wq", [D, DML], FP32, kind="ExternalInput")[:],
        "wk": nc.dram_tensor("wk", [D, DML], FP32, kind="ExternalInput")[:],
        "wv": nc.dram_tensor("wv", [D, DML], FP32, kind="ExternalInput")[:],
        "wo": nc.dram_tensor("wo", [DML, D], FP32, kind="ExternalInput")[:],
        "bq": nc.dram_tensor("bq", [DML], FP32, kind="ExternalInput")[:],
        "bk": nc.dram_tensor("bk", [DML], FP32, kind="ExternalInput")[:],
        "bv": nc.dram_tensor("bv", [DML], FP32, kind="ExternalInput")[:],
        "out": nc.dram_tensor("out", [S, D], FP32, kind="ExternalOutput")[:],
    }
    with tile.TileContext(nc) as tc:
        for _ in range(reps):
            build_attention(tc, io, S, D, DML)
    split_excess_waits(nc)
    return nc


_NC_CACHE = {}


def kernel(**inputs):
    query = np.asarray(inputs["query"], np.float32)
    key = np.asarray(inputs["key"], np.float32)
    value = np.asarray(inputs["value"], np.float32)
    mask = np.asarray(inputs["mask"], np.int32)
    Wq, bq = np.asarray(inputs["Wq"], np.float32), np.asarray(inputs["bq"], np.float32)
    Wk, bk = np.asarray(inputs["Wk"], np.float32), np.asarray(inputs["bk"], np.float32)
    Wv, bv = np.asarray(inputs["Wv"], np.float32), np.asarray(inputs["bv"], np.float32)
    Wo, bo = np.asarray(inputs["Wo"], np.float32), np.asarray(inputs["bo"], np.float32)

    B = query.shape[0]
    DML = Wq.shape[1] // 2  # head-group slice width

    if "nc" not in _NC_CACHE:
        _NC_CACHE["nc"] = build_nc()
    nc = _NC_CACHE["nc"]

    in_maps = []
    for c in range(N_CORES):
        b, g = divmod(c, 2)
        sl = slice(g * DML, (g + 1) * DML)
        in_maps.append({
            "xq": np.ascontiguousarray(query[b]),
            "xk": np.ascontiguousarray(key[b]),
            "xv": np.ascontiguousarray(value[b]),
            "mask": np.ascontiguousarray(mask[b]),
            "wq": np.ascontiguousarray(Wq[:, sl]),
            "wk": np.ascontiguousarray(Wk[:, sl]),
            "wv": np.ascontiguousarray(Wv[:, sl]),
            "wo": np.ascontiguousarray(Wo[sl, :]),
            "bq": np.ascontiguousarray(bq[sl]),
            "bk": np.ascontiguousarray(bk[sl]),
            "bv": np.ascontiguousarray(bv[sl]),
        })

    import os

    from concourse.bass_utils import run_bass_kernel_spmd
    trace = os.environ.get("KERNEL_TRACE", "0") == "1"
    res = run_bass_kernel_spmd(nc, in_maps, core_ids=list(range(N_CORES)), trace=trace)
    _NC_CACHE["last_result"] = res
    out = np.stack([
        res.results[2 * b]["out"] + res.results[2 * b + 1]["out"] + bo
        for b in range(B)
    ]).astype(np.float32)
    return out
